# revision 1
# baseline (speedup 1.0000x reference)
"""Trainium2 Bass kernel for nn_Block_34394098106804 (dense transformer block
with soft-MoE FFN), SPMD over 8 NeuronCores.

Sharding: core = 2*b + g  (b = batch 0..3, g = 0/1).
  Stage 1 (attention): data-parallel over B, tensor-parallel over heads
    (6 heads per core).  Each core computes LN1(x[b]), its heads' q/k/v,
    causal softmax attention and a partial projection  Y_g @ proj_w[cols_g].
  Exchange: pairwise ReduceScatter (bf16), one chunk per 512-token score
    block, issued as soon as that block's projection lands so the
    collective, LN2 and the gate all pipeline under the scores tail.
  Stage 2 (soft-MoE FFN): token-parallel.  Each core runs LN2 + gate +
    all 8 experts for its 1024 tokens, accumulating gate-weighted expert
    outputs into DRAM with accumulate-DMA.  Output is x + attn + moe for
    its rows; the host reassembles the interleaved row blocks.

Matmuls run in bf16 with fp32 PSUM accumulation; softmax/LN statistics and
residuals stay fp32.
"""

import contextlib

import numpy as np
import ml_dtypes

import bass_rust
import concourse.bass as bass
import concourse.mybir as mybir
import concourse.tile as tile
from concourse.tile import add_dep_helper
from concourse.masks import make_identity
from concourse.bass_utils import run_bass_kernel_spmd

B, T, C, H, E = 4, 2048, 768, 12, 8
DFF = 4 * C
HD = C // H
P = 128
NCORES = 8
HPC = H // 2          # heads per core = 6
TH = T // 2           # tokens per core in stage 2 = 1024
CI = C // P           # 6 contraction chunks over C
KO = DFF // P         # 24 dff chunks
NTT = T // P          # 16 token tiles (stage 1)
NTT2 = TH // P        # 8 token tiles (stage 2)
NCHUNK = 4            # ReduceScatter chunks
SCALE = 1.0 / np.sqrt(HD)
PAIRS = [[0, 1], [2, 3], [4, 5], [6, 7]]

F32 = mybir.dt.float32
F32R = mybir.dt.float32r
BF16 = mybir.dt.bfloat16
AX = mybir.AxisListType
ALU = mybir.AluOpType
ACTF = mybir.ActivationFunctionType

_CACHED_NC = None
LAST_RESULT = [None]


def _split_multi_waits(nc):
    """walrus in this toolchain encodes at most one sync wait per
    instruction; hoist extras onto single-wait nops placed just before."""
    count = 0
    for f in nc.m.functions:
        for blk in f.blocks:
            new_insts = []
            changed = False
            for inst in blk.instructions:
                si = inst.sync_info
                if si is not None and si.on_wait and len(si.on_wait) > 1:
                    waits = list(si.on_wait)
                    for w in waits[:-1]:
                        nop = bass_rust.InstNoOp(
                            name=f"I-wsplit-{count}", ins=[], outs=[])
                        count += 1
                        nop.engine = inst.engine
                        nop.sync_info = mybir.SyncInfo(on_wait=[w], on_update=[])
                        new_insts.append(nop)
                    si.on_wait = [waits[-1]]
                    changed = True
                new_insts.append(inst)
            if changed:
                blk.instructions = new_insts
    return count


_IDENT = [None]


def _emit_ln_transpose(nc, pools, x_tile, dst, dst_tok, lnw, lnb, eps):
    """LayerNorm rows of x_tile [128, C] f32 (tokens on partitions), then
    PE-transpose into dst [128, CI, ntok] bf16 at token offset dst_tok,
    applying the per-channel scale/bias in the transposed layout."""
    temps, stats, tpsum = pools
    st = stats.tile([P, 3, 6], F32, tag="ln_st")
    for s in range(3):
        nc.vector.bn_stats(out=st[:, s, :], in_=x_tile[:, s * 256:(s + 1) * 256])
    mv = stats.tile([P, 2], F32, tag="ln_mv")
    nc.vector.bn_aggr(out=mv[:], in_=st[:])
    std = stats.tile([P, 1], F32, tag="ln_std")
    nc.scalar.activation(out=std[:], in_=mv[:, 1:2], func=ACTF.Sqrt,
                         bias=eps[:], scale=1.0)
    rstd = stats.tile([P, 1], F32, tag="ln_rstd")
    nc.vector.reciprocal(out=rstd[:], in_=std[:])
    nrm = temps.tile([P, C], BF16, tag="ln_nrm")
    nc.vector.tensor_scalar(out=nrm[:], in0=x_tile[:], scalar1=mv[:, 0:1],
                            scalar2=rstd[:], op0=ALU.subtract, op1=ALU.mult)
    for ci in range(CI):
        pt = tpsum.tile([P, P], BF16, tag="ln_tp")
        nc.tensor.transpose(pt[:], nrm[:, ci * P:(ci + 1) * P], _IDENT[0])
        nc.vector.tensor_scalar(out=dst[:, ci, dst_tok:dst_tok + P], in0=pt[:],
                                scalar1=lnw[:, ci:ci + 1], scalar2=lnb[:, ci:ci + 1],
                                op0=ALU.mult, op1=ALU.add)


def _build_nc():
    nc = bass.Bass(num_devices=NCORES)

    # ---- DRAM parameters (per-core shards supplied by kernel()) ----
    xb = nc.declare_dram_parameter("xb", [T, C], F32, isOutput=False)
    x_half = nc.declare_dram_parameter("x_half", [TH, C], F32, isOutput=False)
    wq = nc.declare_dram_parameter("wq", [C, HPC * HD], BF16, isOutput=False)
    wk = nc.declare_dram_parameter("wk", [C, HPC * HD], BF16, isOutput=False)
    wv = nc.declare_dram_parameter("wv", [C, HPC * HD], BF16, isOutput=False)
    pw = nc.declare_dram_parameter("pw", [HPC * HD, C], BF16, isOutput=False)
    ln1w = nc.declare_dram_parameter("ln1w", [C], F32, isOutput=False)
    ln1b = nc.declare_dram_parameter("ln1b", [C], F32, isOutput=False)
    ln2w = nc.declare_dram_parameter("ln2w", [C], F32, isOutput=False)
    ln2b = nc.declare_dram_parameter("ln2b", [C], F32, isOutput=False)
    gw = nc.declare_dram_parameter("gw", [C, E], BF16, isOutput=False)
    fcw = nc.declare_dram_parameter("fcw", [E, C, DFF], BF16, isOutput=False)
    fcb = nc.declare_dram_parameter("fcb", [E, DFF], F32, isOutput=False)
    prw = nc.declare_dram_parameter("prw", [E, DFF, C], BF16, isOutput=False)
    out_half = nc.declare_dram_parameter("out_half", [TH, C], F32, isOutput=True)

    # ---- internal DRAM ----
    pp_dram = nc.dram_tensor("pp_dram", [T, C], BF16)
    rs_dram = nc.dram_tensor("rs_dram", [TH, C], BF16)
    moe_dram = nc.dram_tensor("moe_dram", [TH, C], F32)

    with tile.TileContext(nc) as tc, contextlib.ExitStack() as octx:
        singles = octx.enter_context(tc.tile_pool(name="singles", bufs=1))
        ident = singles.tile([P, P], BF16)
        make_identity(nc, ident[:])
        _IDENT[0] = ident
        eps = singles.tile([P, 1], F32)
        nc.vector.memset(eps[:], 1e-5)
        ln2w_sb = singles.tile([P, CI], F32)
        nc.sync.dma_start(out=ln2w_sb[:], in_=ln2w.rearrange("(o p) -> p o", p=P))
        ln2b_sb = singles.tile([P, CI], F32)
        nc.sync.dma_start(out=ln2b_sb[:], in_=ln2b.rearrange("(o p) -> p o", p=P))
        gw_sb = singles.tile([P, CI, E], BF16)
        nc.sync.dma_start(out=gw_sb[:], in_=gw.rearrange("(o p) n -> p o n", p=P))
        ones64_f = singles.tile([1, HD], F32)
        nc.vector.memset(ones64_f[:], 1.0)
        ones64 = singles.tile([1, HD], F32R)
        with nc.allow_low_precision(reason="f32r stores full fp32 bits"):
            nc.vector.tensor_copy(out=ones64[:], in_=ones64_f[:])
        h2T = singles.tile([P, CI, TH], BF16)
        g_sb = singles.tile([P, NTT2, E], F32)

        # shared PSUM pools, live for the whole kernel (8 banks budget):
        #   tp (1) LN transposes, qkp (2) qkv/proj/fc, gtp (1) gate,
        #   scores scope adds s (2) + yl (2); expert scope adds eo (2).
        tpsum = octx.enter_context(tc.tile_pool(name="tp_psum", bufs=1, space="PSUM"))
        qkp = octx.enter_context(tc.tile_pool(name="qk_psum", bufs=2, space="PSUM"))
        gtp = octx.enter_context(tc.tile_pool(name="gt_psum", bufs=1, space="PSUM"))

        # stage-2 weight/hidden pools hoisted to kernel scope so expert-0's
        # fc weights DMA and gelu writes don't wait for stage-1 SBUF reuse
        wfp = octx.enter_context(tc.tile_pool(name="wf", bufs=1))
        hidp = octx.enter_context(tc.tile_pool(name="hid", bufs=1))
        fcbp = octx.enter_context(tc.tile_pool(name="fcb", bufs=2))
        stats2 = octx.enter_context(tc.tile_pool(name="s2_stats", bufs=4))
        temps6 = octx.enter_context(tc.tile_pool(name="s2_temps", bufs=2))

        accum_tail = {}
        ccs = []

        # ================= stage 1 (+ pipelined exchange & LN2/gate) ========
        with contextlib.ExitStack() as s1:
            s1s = s1.enter_context(tc.tile_pool(name="s1_singles", bufs=1))
            temps = s1.enter_context(tc.tile_pool(name="s1_temps", bufs=2))
            stats = s1.enter_context(tc.tile_pool(name="s1_stats", bufs=4))

            ln1w_sb = s1s.tile([P, CI], F32)
            nc.sync.dma_start(out=ln1w_sb[:], in_=ln1w.rearrange("(o p) -> p o", p=P))
            ln1b_sb = s1s.tile([P, CI], F32)
            nc.sync.dma_start(out=ln1b_sb[:], in_=ln1b.rearrange("(o p) -> p o", p=P))
            wq_sb = s1s.tile([P, CI, HPC * HD], BF16)
            nc.sync.dma_start(out=wq_sb[:], in_=wq.rearrange("(o p) n -> p o n", p=P))
            wk_sb = s1s.tile([P, CI, HPC * HD], BF16)
            nc.sync.dma_start(out=wk_sb[:], in_=wk.rearrange("(o p) n -> p o n", p=P))
            wv_sb = s1s.tile([P, CI, HPC * HD], BF16)
            nc.sync.dma_start(out=wv_sb[:], in_=wv.rearrange("(o p) n -> p o n", p=P))
            pw_sb = s1s.tile([P, 3, C], BF16)
            nc.sync.dma_start(out=pw_sb[:], in_=pw.rearrange("(o p) n -> p o n", p=P))
            masks = s1s.tile([P, 4, 512], BF16)
            nc.vector.memset(masks[:], 1.0)
            for d in range(4):
                nc.gpsimd.affine_select(
                    out=masks[:, d, :], in_=masks[:, d, :],
                    pattern=[[1, 512]], channel_multiplier=-1,
                    base=-(d * P), compare_op=ALU.is_ge, fill=0.0)

            hT = s1s.tile([P, CI, T], BF16)
            q_sb = s1s.tile([P, 3, T], BF16)   # 2 heads per 128 partitions
            k_sb = s1s.tile([P, 3, T], BF16)
            v_sb = s1s.tile([P, NTT, HPC, HD + 1], BF16)  # [tok, head, hd|1]
            nc.vector.memset(v_sb[:, :, :, HD:HD + 1], 1.0)
            y_sb = s1s.tile([P, 3, T], BF16)

            # --- LN1 + transpose ---
            for ti in range(NTT):
                xt = temps.tile([P, C], BF16, tag="xt")
                nc.gpsimd.dma_start(out=xt[:], in_=xb[ti * P:(ti + 1) * P, :])
                _emit_ln_transpose(nc, (temps, stats, tpsum), xt, hT,
                                   ti * P, ln1w_sb, ln1b_sb, eps)

            # --- qkv ---
            for j in range(3):
                for tb in range(4):
                    for (w_sb, dst) in ((wq_sb, q_sb), (wk_sb, k_sb)):
                        pq = qkp.tile([P, 512], F32, tag="qk")
                        for ci in range(CI):
                            nc.tensor.matmul(
                                pq[:], w_sb[:, ci, j * P:(j + 1) * P],
                                hT[:, ci, tb * 512:(tb + 1) * 512],
                                start=(ci == 0), stop=(ci == CI - 1))
                        nc.any.tensor_copy(
                            out=dst[:, j, tb * 512:(tb + 1) * 512], in_=pq[:])
            for ti in range(NTT):
                pv = qkp.tile([P, 512], F32, tag="qk")
                for ci in range(CI):
                    nc.tensor.matmul(
                        pv[:, :HPC * HD], hT[:, ci, ti * P:(ti + 1) * P],
                        wv_sb[:, ci, :],
                        start=(ci == 0), stop=(ci == CI - 1))
                nc.any.tensor_copy(
                    out=v_sb[:, ti, :, 0:HD],
                    in_=pv[:, :HPC * HD].rearrange("p (h d) -> p h d", h=HPC))

            # --- per 512-token block: scores/AV -> proj -> RS -> LN2/gate ---
            with contextlib.ExitStack() as s3:
                spool = s3.enter_context(
                    tc.tile_pool(name="s_psum", bufs=2, space="PSUM"))
                ylpool = s3.enter_context(
                    tc.tile_pool(name="yl_psum", bufs=2, space="PSUM"))
                expool = s3.enter_context(tc.tile_pool(name="exps", bufs=3))
                lrow = s3.enter_context(tc.tile_pool(name="lrow", bufs=2))
                for tqb in range(4):
                    ntk = (tqb + 1) * 4

                    def attend(h):
                        """scores + exp + AV for one head; returns the yl psum."""
                        off = HD * (h % 2)
                        j = h // 2
                        yl = ylpool.tile([HD + 1, 512], F32, tag="yl")
                        for tki in range(ntk):
                            ps = spool.tile([P, 512], F32, tag="s")
                            nc.tensor.matmul(
                                ps[:], k_sb[off:off + HD, j, tki * P:(tki + 1) * P],
                                q_sb[off:off + HD, j, tqb * 512:(tqb + 1) * 512],
                                start=True, stop=True)
                            ex = expool.tile([P, 512], BF16, tag="ex")
                            nc.scalar.activation(out=ex[:], in_=ps[:],
                                                 func=ACTF.Exp, scale=SCALE)
                            d = tki - tqb * 4
                            if d >= 0:
                                nc.vector.tensor_mul(
                                    out=ex[:], in0=ex[:], in1=masks[:, d, :])
                            nc.tensor.matmul(
                                yl[:], v_sb[:, tki, h, :], ex[:],
                                start=(tki == 0), stop=(tki == ntk - 1))
                        # 1/l as exp(-ln(l)) -- two fast ACT LUT passes
                        # instead of DVE reciprocal (4us on a [1,512] row)
                        rf = lrow.tile([1, 512], F32, tag="rf")
                        nc.scalar.activation(out=rf[:], in_=yl[HD:HD + 1, :],
                                             func=ACTF.Ln)
                        rl = lrow.tile([1, 512], F32R, tag="rl")
                        nc.scalar.activation(out=rl[:], in_=rf[:],
                                             func=ACTF.Exp, scale=-1.0)
                        return yl, rl

                    def finalize(h, yl, rl):
                        """broadcast 1/l across partitions and normalize yT."""
                        off = HD * (h % 2)
                        j = h // 2
                        gpb = spool.tile([P, 512], F32, tag="s")
                        nc.tensor.matmul(gpb[:HD, :], ones64[:], rl[:],
                                         start=True, stop=True)
                        gs = lrow.tile([HD, 512], F32, tag="gs")
                        nc.any.tensor_copy(out=gs[:], in_=gpb[:HD, :])
                        nc.vector.tensor_mul(
                            out=y_sb[off:off + HD, j, tqb * 512:(tqb + 1) * 512],
                            in0=yl[0:HD, :], in1=gs[:])

                    # pair-batched: head h+1's matmuls hide head h's reciprocal
                    for hp in range(HPC // 2):
                        a = attend(2 * hp)
                        b_ = attend(2 * hp + 1)
                        finalize(2 * hp, *a)
                        finalize(2 * hp + 1, *b_)

                    # projection for this block's 4 token tiles (psum: qkp)
                    chunk_writes = []
                    for ti in range(tqb * 4, tqb * 4 + 4):
                        po = temps.tile([P, C], BF16, tag="po")
                        for nh in range(2):
                            pp = qkp.tile([P, 512], F32, tag="qk")
                            for j in range(3):
                                nc.tensor.matmul(
                                    pp[:, :384],
                                    y_sb[:, j, ti * P:(ti + 1) * P],
                                    pw_sb[:, j, nh * 384:(nh + 1) * 384],
                                    start=(j == 0), stop=(j == 2))
                            nc.any.tensor_copy(out=po[:, nh * 384:(nh + 1) * 384],
                                               in_=pp[:, :384])
                        w = nc.sync.dma_start(out=pp_dram[ti * P:(ti + 1) * P, :],
                                              in_=po[:])
                        chunk_writes.append(w)

                    k = tqb
                    cc = nc.gpsimd.collective_compute(
                        "ReduceScatter", ALU.add, replica_groups=PAIRS,
                        ins=[pp_dram[k * 512:(k + 1) * 512, :]],
                        outs=[rs_dram[k * 256:(k + 1) * 256, :]])
                    for w in chunk_writes:
                        add_dep_helper(cc.ins, w.ins, sync=True,
                                       reason="cc after pp writes")
                    ccs.append(cc)

            # scheduler fence: keep every LN2/expert instruction after the
            # whole scores/proj region in each engine's static order, so a
            # slow pair-peer's collective can't block the scores pipeline
            tc.no_sync_barrier()

            # --- LN2 + gate, per RS chunk (after scores: no PE stalls) ---
            for k in range(NCHUNK):
                for m in range(2):
                    ti2 = 2 * k + m
                    rst = temps6.tile([P, C], BF16, tag="rst")
                    r = nc.sync.dma_start(
                        out=rst[:], in_=rs_dram[ti2 * P:(ti2 + 1) * P, :])
                    add_dep_helper(r.ins, ccs[k].ins, sync=True,
                                   reason="read rs chunk")
                    xa = temps6.tile([P, C], F32, tag="xa")
                    nc.sync.dma_start(out=xa[:],
                                      in_=x_half[ti2 * P:(ti2 + 1) * P, :])
                    nc.vector.tensor_add(out=xa[:], in0=xa[:], in1=rst[:])
                    aw = nc.sync.dma_start(
                        out=moe_dram[ti2 * P:(ti2 + 1) * P, :], in_=xa[:])
                    accum_tail[ti2] = aw
                    _emit_ln_transpose(nc, (temps6, stats2, tpsum), xa, h2T,
                                       ti2 * P, ln2w_sb, ln2b_sb, eps)
                    gp2 = gtp.tile([P, E], F32, tag="gt")
                    for ci in range(CI):
                        nc.tensor.matmul(gp2[:],
                                         h2T[:, ci, ti2 * P:(ti2 + 1) * P],
                                         gw_sb[:, ci, :],
                                         start=(ci == 0), stop=(ci == CI - 1))
                    nmax = stats2.tile([P, 1], F32, tag="gmax")
                    nc.vector.tensor_reduce(out=nmax[:], in_=gp2[:], axis=AX.X,
                                            op=ALU.max, negate=True)
                    ge = stats2.tile([P, E], F32, tag="gexp")
                    nc.scalar.activation(out=ge[:], in_=gp2[:], func=ACTF.Exp,
                                         bias=nmax[:], scale=1.0)
                    gsum = stats2.tile([P, 1], F32, tag="gsum")
                    nc.vector.reduce_sum(out=gsum[:], in_=ge[:], axis=AX.X)
                    grec = stats2.tile([P, 1], F32, tag="grec")
                    nc.vector.reciprocal(out=grec[:], in_=gsum[:])
                    nc.vector.tensor_scalar_mul(g_sb[:, ti2, :], ge[:], grec[:])

        # ================= stage 2: experts =================
        with contextlib.ExitStack() as s2:
            gtp2 = s2.enter_context(tc.tile_pool(name="s2_gt", bufs=2))
            wpp = s2.enter_context(tc.tile_pool(name="wp", bufs=2))
            eps_ = s2.enter_context(
                tc.tile_pool(name="e_psum", bufs=4, space="PSUM"))

            for e in range(E):
                wf = wfp.tile([P, CI, DFF], BF16, tag="wf")
                nc.sync.dma_start(
                    out=wf[:], in_=fcw[e].rearrange("(o p) f -> p o f", p=P))
                wp = wpp.tile([P, KO, C], BF16, tag="wp")
                nc.sync.dma_start(
                    out=wp[:], in_=prw[e].rearrange("(o p) f -> p o f", p=P))
                fb = fcbp.tile([P, KO], F32, tag="fcb")
                nc.sync.dma_start(
                    out=fb[:], in_=fcb[e].rearrange("(o p) -> p o", p=P))
                for th in range(2):
                    hid = hidp.tile([P, KO, 512], BF16, tag="hid")
                    for ko in range(KO):
                        fp = qkp.tile([P, 512], F32, tag="qk")
                        for ci in range(CI):
                            nc.tensor.matmul(
                                fp[:], wf[:, ci, ko * P:(ko + 1) * P],
                                h2T[:, ci, th * 512:(th + 1) * 512],
                                start=(ci == 0), stop=(ci == CI - 1))
                        nc.scalar.activation(
                            out=hid[:, ko, :], in_=fp[:],
                            func=ACTF.Gelu, bias=fb[:, ko:ko + 1])
                    for tl in range(4):
                        ti = th * 4 + tl
                        gt = gtp2.tile([P, C], F32, tag="gt2")
                        for nh in range(2):
                            ep = eps_.tile([P, 384], F32, tag="ep")
                            for ko in range(KO):
                                nc.tensor.matmul(
                                    ep[:], hid[:, ko, tl * P:(tl + 1) * P],
                                    wp[:, ko, nh * 384:(nh + 1) * 384],
                                    start=(ko == 0), stop=(ko == KO - 1))
                            nc.vector.tensor_scalar_mul(
                                gt[:, nh * 384:(nh + 1) * 384], ep[:],
                                g_sb[:, ti, e:e + 1])
                        aw = nc.gpsimd.dma_start(
                            out=moe_dram[ti * P:(ti + 1) * P, :], in_=gt[:],
                            accum_op=ALU.add)
                        add_dep_helper(aw.ins, accum_tail[ti].ins, sync=True,
                                       reason="serialize accum")
                        accum_tail[ti] = aw

            od = nc.sync.dma_start(out=out_half[:], in_=moe_dram[:])
            for ti in range(NTT2):
                add_dep_helper(od.ins, accum_tail[ti].ins, sync=True,
                               reason="out after accum")

    _split_multi_waits(nc)
    return nc


def _get_nc():
    global _CACHED_NC
    if _CACHED_NC is None:
        _CACHED_NC = _build_nc()
    return _CACHED_NC


def _row_blocks(g):
    """Global row blocks (within a batch) owned by pair-rank g, in the local
    order produced by the chunked ReduceScatter."""
    return [slice(k * 512 + g * 256, k * 512 + (g + 1) * 256)
            for k in range(NCHUNK)]


def kernel(**inputs):
    np_in = {k: np.asarray(v) for k, v in inputs.items()}
    bf = lambda a: np.ascontiguousarray(a).astype(ml_dtypes.bfloat16)
    f32 = lambda a: np.ascontiguousarray(a, dtype=np.float32)

    x = f32(np_in["x"])
    attn_w = np_in["attn_w"]
    fcw = bf(np_in["fc_w"])
    prw = bf(np_in["pr_w"])
    fcb = f32(np_in["fc_b"])
    gw = bf(np_in["gate_w"])
    ln1w, ln1b = f32(np_in["ln1_w"]), f32(np_in["ln1_b"])
    ln2w, ln2b = f32(np_in["ln2_w"]), f32(np_in["ln2_b"])
    proj_w = np_in["proj_w"]

    in_maps = []
    for core in range(NCORES):
        b, g = core // 2, core % 2
        cols = slice(g * HPC * HD, (g + 1) * HPC * HD)
        in_maps.append({
            "xb": x[b],
            "x_half": np.concatenate([x[b, blk] for blk in _row_blocks(g)]),
            "wq": bf(attn_w[:, cols]),
            "wk": bf(attn_w[:, C:2 * C][:, cols]),
            "wv": bf(attn_w[:, 2 * C:3 * C][:, cols]),
            "pw": bf(proj_w[cols, :]),
            "ln1w": ln1w, "ln1b": ln1b, "ln2w": ln2w, "ln2b": ln2b,
            "gw": gw, "fcw": fcw, "fcb": fcb, "prw": prw,
        })

    nc = _get_nc()
    res = run_bass_kernel_spmd(nc, in_maps, core_ids=list(range(NCORES)))
    LAST_RESULT[0] = res

    out = np.empty((B, T, C), dtype=np.float32)
    for core in range(NCORES):
        b, g = core // 2, core % 2
        oh = res.results[core]["out_half"]
        for k, blk in enumerate(_row_blocks(g)):
            out[b, blk] = oh[k * 256:(k + 1) * 256]
    return out



# revision 4
# speedup vs baseline: 1.2799x; 1.2799x over previous
"""Trainium2 Bass kernel for nn_Block_34394098106804 (dense transformer block
with soft-MoE FFN), SPMD over 8 NeuronCores.

Sharding: core = 2*b + g  (b = batch 0..3, g = 0/1).
  Stage 1 (attention): data-parallel over B, tensor-parallel over heads
    (6 heads per core).  Scores and AV are decoupled per (query-block, head
    pair): both heads' score matmuls are issued adjacently so they run
    concurrently in disjoint PE row groups (K=64 each), exp() runs as one
    paired ACT over a 2-bank PSUM tile into an SBUF buffer, then the AV
    matmuls stream back-to-back.  Projection partials go out through a
    pairwise chunked ReduceScatter that pipelines with LN2 and the gate.
  Stage 2 (soft-MoE FFN): token-parallel, all 8 experts per core for its
    1024 tokens.  fc runs in bf16; the pr matmul runs in fp8-e4m3 with
    perf_mode=DoubleRow (2 K-blocks per pass), with pr_w pre-scaled by 16
    on the host and the gate scaled by 1/16 to compensate.  Expert outputs
    accumulate into DRAM with accumulate-DMA.

Matmuls in bf16 (fp8 for pr) with fp32 PSUM accumulation; softmax/LN
statistics and residuals stay fp32.
"""

import contextlib

import numpy as np
import ml_dtypes

import bass_rust
import concourse.bass as bass
import concourse.mybir as mybir
import concourse.tile as tile
from concourse.tile import add_dep_helper
from concourse.masks import make_identity
from concourse.bass_utils import run_bass_kernel_spmd

B, T, C, H, E = 4, 2048, 768, 12, 8
DFF = 4 * C
HD = C // H
P = 128
NCORES = 8
HPC = H // 2          # heads per core = 6
TH = T // 2           # tokens per core in stage 2 = 1024
CI = C // P           # 6 contraction chunks over C
KO = DFF // P         # 24 dff chunks
KO2 = KO // 2         # 12 dff pair-chunks (fp8 DoubleRow)
NTT = T // P          # 16 token tiles (stage 1)
NTT2 = TH // P        # 8 token tiles (stage 2)
NCHUNK = 4            # ReduceScatter chunks
SCALE = 1.0 / np.sqrt(HD)
WS = 16.0             # host-side pre-scale on pr_w (fp8 subnormal avoidance)
PAIRS = [[0, 1], [2, 3], [4, 5], [6, 7]]

F32 = mybir.dt.float32
F32R = mybir.dt.float32r
BF16 = mybir.dt.bfloat16
FP8 = mybir.dt.float8e4
AX = mybir.AxisListType
ALU = mybir.AluOpType
ACTF = mybir.ActivationFunctionType
DR = mybir.MatmulPerfMode.DoubleRow

_CACHED_NC = None
LAST_RESULT = [None]


def _split_multi_waits(nc):
    """walrus in this toolchain encodes at most one sync wait per
    instruction; hoist extras onto single-wait nops placed just before."""
    count = 0
    for f in nc.m.functions:
        for blk in f.blocks:
            new_insts = []
            changed = False
            for inst in blk.instructions:
                si = inst.sync_info
                if si is not None and si.on_wait and len(si.on_wait) > 1:
                    waits = list(si.on_wait)
                    for w in waits[:-1]:
                        nop = bass_rust.InstNoOp(
                            name=f"I-wsplit-{count}", ins=[], outs=[])
                        count += 1
                        nop.engine = inst.engine
                        nop.sync_info = mybir.SyncInfo(on_wait=[w], on_update=[])
                        new_insts.append(nop)
                    si.on_wait = [waits[-1]]
                    changed = True
                new_insts.append(inst)
            if changed:
                blk.instructions = new_insts
    return count


_IDENT = [None]


def _emit_ln_transpose(nc, pools, x_tile, dst, dst_tok, lnw, lnb, eps):
    """LayerNorm rows of x_tile [128, C] f32 (tokens on partitions), then
    PE-transpose into dst [128, CI, ntok] bf16 at token offset dst_tok,
    applying the per-channel scale/bias in the transposed layout."""
    temps, stats, tpsum = pools
    st = stats.tile([P, 3, 6], F32, tag="ln_st")
    for s in range(3):
        nc.vector.bn_stats(out=st[:, s, :], in_=x_tile[:, s * 256:(s + 1) * 256])
    mv = stats.tile([P, 2], F32, tag="ln_mv")
    nc.vector.bn_aggr(out=mv[:], in_=st[:])
    std = stats.tile([P, 1], F32, tag="ln_std")
    nc.scalar.activation(out=std[:], in_=mv[:, 1:2], func=ACTF.Sqrt,
                         bias=eps[:], scale=1.0)
    rstd = stats.tile([P, 1], F32, tag="ln_rstd")
    nc.vector.reciprocal(out=rstd[:], in_=std[:])
    nrm = temps.tile([P, C], BF16, tag="ln_nrm")
    nc.vector.tensor_scalar(out=nrm[:], in0=x_tile[:], scalar1=mv[:, 0:1],
                            scalar2=rstd[:], op0=ALU.subtract, op1=ALU.mult)
    for ci in range(CI):
        pt = tpsum.tile([P, P], BF16, tag="ln_tp")
        nc.tensor.transpose(pt[:], nrm[:, ci * P:(ci + 1) * P], _IDENT[0])
        nc.vector.tensor_scalar(out=dst[:, ci, dst_tok:dst_tok + P], in0=pt[:],
                                scalar1=lnw[:, ci:ci + 1], scalar2=lnb[:, ci:ci + 1],
                                op0=ALU.mult, op1=ALU.add)


def _build_nc():
    nc = bass.Bass(num_devices=NCORES)

    # ---- DRAM parameters (per-core shards supplied by kernel()) ----
    xb = nc.declare_dram_parameter("xb", [T, C], F32, isOutput=False)
    x_half = nc.declare_dram_parameter("x_half", [TH, C], F32, isOutput=False)
    wq = nc.declare_dram_parameter("wq", [C, HPC * HD], BF16, isOutput=False)
    wk = nc.declare_dram_parameter("wk", [C, HPC * HD], BF16, isOutput=False)
    wv = nc.declare_dram_parameter("wv", [C, HPC * HD], BF16, isOutput=False)
    pw = nc.declare_dram_parameter("pw", [HPC * HD, C], BF16, isOutput=False)
    ln1w = nc.declare_dram_parameter("ln1w", [C], F32, isOutput=False)
    ln1b = nc.declare_dram_parameter("ln1b", [C], F32, isOutput=False)
    ln2w = nc.declare_dram_parameter("ln2w", [C], F32, isOutput=False)
    ln2b = nc.declare_dram_parameter("ln2b", [C], F32, isOutput=False)
    gw = nc.declare_dram_parameter("gw", [C, E], BF16, isOutput=False)
    fcw = nc.declare_dram_parameter("fcw", [E, C, DFF], BF16, isOutput=False)
    fcb = nc.declare_dram_parameter("fcb", [E, DFF], F32, isOutput=False)
    prw = nc.declare_dram_parameter("prw", [E, DFF, C], FP8, isOutput=False)
    out_half = nc.declare_dram_parameter("out_half", [TH, C], F32, isOutput=True)

    # ---- internal DRAM ----
    pp_dram = nc.dram_tensor("pp_dram", [T, C], BF16)
    rs_dram = nc.dram_tensor("rs_dram", [TH, C], BF16)
    moe_dram = nc.dram_tensor("moe_dram", [TH, C], F32)

    with tile.TileContext(nc) as tc, contextlib.ExitStack() as octx:
        singles = octx.enter_context(tc.tile_pool(name="singles", bufs=1))
        ident = singles.tile([P, P], BF16)
        make_identity(nc, ident[:])
        _IDENT[0] = ident
        eps = singles.tile([P, 1], F32)
        nc.vector.memset(eps[:], 1e-5)
        ln2w_sb = singles.tile([P, CI], F32)
        nc.sync.dma_start(out=ln2w_sb[:], in_=ln2w.rearrange("(o p) -> p o", p=P))
        ln2b_sb = singles.tile([P, CI], F32)
        nc.sync.dma_start(out=ln2b_sb[:], in_=ln2b.rearrange("(o p) -> p o", p=P))
        gw_sb = singles.tile([P, CI, E], BF16)
        nc.sync.dma_start(out=gw_sb[:], in_=gw.rearrange("(o p) n -> p o n", p=P))
        ones64_f = singles.tile([1, HD], F32)
        nc.vector.memset(ones64_f[:], 1.0)
        ones64 = singles.tile([1, HD], F32R)
        with nc.allow_low_precision(reason="f32r stores full fp32 bits"):
            nc.vector.tensor_copy(out=ones64[:], in_=ones64_f[:])
        h2T = singles.tile([P, CI, TH], BF16)
        g_sb = singles.tile([P, NTT2, E], F32)
        masks = singles.tile([P, 4, 512], BF16)
        nc.vector.memset(masks[:], 1.0)
        for d in range(4):
            nc.gpsimd.affine_select(
                out=masks[:, d, :], in_=masks[:, d, :],
                pattern=[[1, 512]], channel_multiplier=-1,
                base=-(d * P), compare_op=ALU.is_ge, fill=0.0)

        # transpose PSUM pool, live for the whole kernel; also hosts the
        # tiny gate psum tile.
        tpsum = octx.enter_context(tc.tile_pool(name="tp_psum", bufs=1, space="PSUM"))

        # stage-2 weight pools hoisted to kernel scope so expert-0's weight
        # DMAs issue early and don't wait for stage-1 SBUF reuse
        wfp = octx.enter_context(tc.tile_pool(name="wf", bufs=2))
        wpp = octx.enter_context(tc.tile_pool(name="wp", bufs=2))
        fcbp = octx.enter_context(tc.tile_pool(name="fcb", bufs=2))
        stats2 = octx.enter_context(tc.tile_pool(name="s2_stats", bufs=4))
        temps6 = octx.enter_context(tc.tile_pool(name="s2_temps", bufs=2))

        accum_tail = {}
        ccs = []

        # ================= stage 1 (+ pipelined exchange & LN2/gate) ========
        with contextlib.ExitStack() as s1:
            s1s = s1.enter_context(tc.tile_pool(name="s1_singles", bufs=1))

            ln1w_sb = s1s.tile([P, CI], F32)
            nc.sync.dma_start(out=ln1w_sb[:], in_=ln1w.rearrange("(o p) -> p o", p=P))
            ln1b_sb = s1s.tile([P, CI], F32)
            nc.sync.dma_start(out=ln1b_sb[:], in_=ln1b.rearrange("(o p) -> p o", p=P))
            pw_sb = s1s.tile([P, 3, C], BF16)
            nc.sync.dma_start(out=pw_sb[:], in_=pw.rearrange("(o p) n -> p o n", p=P))

            # qy_sb: holds q during scores, overwritten head-block by
            # head-block with the normalized attention output y.
            qy_sb = s1s.tile([P, 3, T], BF16)   # 2 heads per 128 partitions
            k_sb = s1s.tile([P, 3, T], BF16)
            v_sb = s1s.tile([P, NTT, HPC, HD + 1], BF16)  # [tok, head, hd|1]
            nc.vector.memset(v_sb[:, :, :, HD:HD + 1], 1.0)

            # --- LN1 + transpose + qkv (own scope: hT/qkp free afterwards) ---
            with contextlib.ExitStack() as s1a:
                s1as = s1a.enter_context(tc.tile_pool(name="s1a_singles", bufs=1))
                temps = s1a.enter_context(tc.tile_pool(name="s1_temps", bufs=2))
                stats = s1a.enter_context(tc.tile_pool(name="s1_stats", bufs=4))
                qkp = s1a.enter_context(
                    tc.tile_pool(name="qk_psum", bufs=2, space="PSUM"))

                wq_sb = s1as.tile([P, CI, HPC * HD], BF16)
                nc.sync.dma_start(out=wq_sb[:], in_=wq.rearrange("(o p) n -> p o n", p=P))
                wk_sb = s1as.tile([P, CI, HPC * HD], BF16)
                nc.sync.dma_start(out=wk_sb[:], in_=wk.rearrange("(o p) n -> p o n", p=P))
                wv_sb = s1as.tile([P, CI, HPC * HD], BF16)
                nc.sync.dma_start(out=wv_sb[:], in_=wv.rearrange("(o p) n -> p o n", p=P))
                hT = s1as.tile([P, CI, T], BF16)

                for ti in range(NTT):
                    xt = temps.tile([P, C], BF16, tag="xt")
                    nc.gpsimd.dma_start(out=xt[:], in_=xb[ti * P:(ti + 1) * P, :])
                    _emit_ln_transpose(nc, (temps, stats, tpsum), xt, hT,
                                       ti * P, ln1w_sb, ln1b_sb, eps)

                for j in range(3):
                    for tb in range(4):
                        for (w_sb, dst) in ((wq_sb, qy_sb), (wk_sb, k_sb)):
                            pq = qkp.tile([P, 512], F32, tag="qk")
                            for ci in range(CI):
                                nc.tensor.matmul(
                                    pq[:], w_sb[:, ci, j * P:(j + 1) * P],
                                    hT[:, ci, tb * 512:(tb + 1) * 512],
                                    start=(ci == 0), stop=(ci == CI - 1))
                            nc.any.tensor_copy(
                                out=dst[:, j, tb * 512:(tb + 1) * 512], in_=pq[:])
                for ti in range(NTT):
                    pv = qkp.tile([P, 512], F32, tag="qk")
                    for ci in range(CI):
                        nc.tensor.matmul(
                            pv[:, :HPC * HD], hT[:, ci, ti * P:(ti + 1) * P],
                            wv_sb[:, ci, :],
                            start=(ci == 0), stop=(ci == CI - 1))
                    nc.any.tensor_copy(
                        out=v_sb[:, ti, :, 0:HD],
                        in_=pv[:, :HPC * HD].rearrange("p (h d) -> p h d", h=HPC))

            # --- per 512-token block: scores -> AV -> proj -> RS -> LN2 ---
            with contextlib.ExitStack() as s3:
                spool = s3.enter_context(
                    tc.tile_pool(name="s_psum", bufs=2, space="PSUM"))
                ylpool = s3.enter_context(
                    tc.tile_pool(name="yl_psum", bufs=1, space="PSUM"))
                expool = s3.enter_context(tc.tile_pool(name="exs", bufs=1))
                lrow = s3.enter_context(tc.tile_pool(name="lrow", bufs=2))
                ex_sb = expool.tile([P, NTT, 2, 512], BF16)

                for tqb in range(4):
                    ntk = (tqb + 1) * 4
                    for hp in range(HPC // 2):
                        j = hp
                        # scores: both heads adjacent -> disjoint row groups
                        for tki in range(ntk):
                            ps2 = spool.tile([P, 2, 512], F32, tag="s")
                            for hh in range(2):
                                off = HD * hh
                                nc.tensor.matmul(
                                    ps2[:, hh, :],
                                    k_sb[off:off + HD, j, tki * P:(tki + 1) * P],
                                    qy_sb[off:off + HD, j, tqb * 512:(tqb + 1) * 512],
                                    start=True, stop=True)
                            nc.scalar.activation(
                                out=ex_sb[:, tki, :, :], in_=ps2[:, :, :],
                                func=ACTF.Exp, scale=SCALE)
                            d = tki - tqb * 4
                            if d >= 0:
                                for hh in range(2):
                                    nc.vector.tensor_mul(
                                        out=ex_sb[:, tki, hh, :],
                                        in0=ex_sb[:, tki, hh, :],
                                        in1=masks[:, d, :])
                        # AV: dense accumulation, both heads interleaved
                        yl2 = ylpool.tile([HD + 1, 2, 512], F32, tag="yl")
                        for tki in range(ntk):
                            for hh in range(2):
                                nc.tensor.matmul(
                                    yl2[:, hh, :], v_sb[:, tki, 2 * hp + hh, :],
                                    ex_sb[:, tki, hh, :],
                                    start=(tki == 0), stop=(tki == ntk - 1))
                        # finalize: 1/l as exp(-ln(l)), paired across heads
                        rf = lrow.tile([1, 2, 512], F32, tag="rf")
                        nc.scalar.activation(out=rf[:], in_=yl2[HD:HD + 1, :, :],
                                             func=ACTF.Ln)
                        rl = lrow.tile([1, 2, 512], F32R, tag="rl")
                        nc.scalar.activation(out=rl[:], in_=rf[:],
                                             func=ACTF.Exp, scale=-1.0)
                        gpb = spool.tile([P, 2, 512], F32, tag="s")
                        for hh in range(2):
                            nc.tensor.matmul(gpb[:HD, hh, :], ones64[:],
                                             rl[0:1, hh, :], start=True, stop=True)
                        gs = lrow.tile([HD, 2, 512], F32, tag="gs")
                        nc.any.tensor_copy(out=gs[:], in_=gpb[:HD, :, :])
                        for hh in range(2):
                            off = HD * hh
                            nc.vector.tensor_mul(
                                out=qy_sb[off:off + HD, j,
                                          tqb * 512:(tqb + 1) * 512],
                                in0=yl2[0:HD, hh, :], in1=gs[:, hh, :])

                    # projection for this block's 4 token tiles
                    chunk_writes = []
                    for ti in range(tqb * 4, tqb * 4 + 4):
                        po = temps6.tile([P, C], BF16, tag="po")
                        ps2p = spool.tile([P, 2, 512], F32, tag="s")
                        for nh in range(2):
                            for jj in range(3):
                                nc.tensor.matmul(
                                    ps2p[:, nh, :384],
                                    qy_sb[:, jj, ti * P:(ti + 1) * P],
                                    pw_sb[:, jj, nh * 384:(nh + 1) * 384],
                                    start=(jj == 0), stop=(jj == 2))
                        nc.any.tensor_copy(out=po[:], in_=ps2p[:, :, :384])
                        w = nc.sync.dma_start(out=pp_dram[ti * P:(ti + 1) * P, :],
                                              in_=po[:])
                        chunk_writes.append(w)

                    k = tqb
                    cc = nc.gpsimd.collective_compute(
                        "ReduceScatter", ALU.add, replica_groups=PAIRS,
                        ins=[pp_dram[k * 512:(k + 1) * 512, :]],
                        outs=[rs_dram[k * 256:(k + 1) * 256, :]])
                    for w in chunk_writes:
                        add_dep_helper(cc.ins, w.ins, sync=True,
                                       reason="cc after pp writes")
                    ccs.append(cc)

                # scheduler fence: keep every LN2/expert instruction after the
                # whole scores/proj region in each engine's static order, so a
                # slow pair-peer's collective can't block the scores pipeline
                tc.no_sync_barrier()

                # --- LN2 + gate, per RS chunk ---
                for k in range(NCHUNK):
                    for m in range(2):
                        ti2 = 2 * k + m
                        rst = temps6.tile([P, C], BF16, tag="rst")
                        r = nc.sync.dma_start(
                            out=rst[:], in_=rs_dram[ti2 * P:(ti2 + 1) * P, :])
                        add_dep_helper(r.ins, ccs[k].ins, sync=True,
                                       reason="read rs chunk")
                        xa = temps6.tile([P, C], F32, tag="xa")
                        nc.sync.dma_start(out=xa[:],
                                          in_=x_half[ti2 * P:(ti2 + 1) * P, :])
                        nc.vector.tensor_add(out=xa[:], in0=xa[:], in1=rst[:])
                        aw = nc.sync.dma_start(
                            out=moe_dram[ti2 * P:(ti2 + 1) * P, :], in_=xa[:])
                        accum_tail[ti2] = aw
                        _emit_ln_transpose(nc, (temps6, stats2, tpsum), xa, h2T,
                                           ti2 * P, ln2w_sb, ln2b_sb, eps)
                        gp2 = tpsum.tile([P, E], F32, tag="gt")
                        for ci in range(CI):
                            nc.tensor.matmul(gp2[:],
                                             h2T[:, ci, ti2 * P:(ti2 + 1) * P],
                                             gw_sb[:, ci, :],
                                             start=(ci == 0), stop=(ci == CI - 1))
                        nmax = stats2.tile([P, 1], F32, tag="gmax")
                        nc.vector.tensor_reduce(out=nmax[:], in_=gp2[:], axis=AX.X,
                                                op=ALU.max, negate=True)
                        ge = stats2.tile([P, E], F32, tag="gexp")
                        nc.scalar.activation(out=ge[:], in_=gp2[:], func=ACTF.Exp,
                                             bias=nmax[:], scale=1.0)
                        gsum = stats2.tile([P, 1], F32, tag="gsum")
                        nc.vector.reduce_sum(out=gsum[:], in_=ge[:], axis=AX.X)
                        grec = stats2.tile([P, 1], F32, tag="grec")
                        nc.vector.reciprocal(out=grec[:], in_=gsum[:])
                        # fold the 1/WS fp8 weight-prescale compensation in
                        nc.vector.tensor_scalar(
                            out=g_sb[:, ti2, :], in0=ge[:], scalar1=grec[:],
                            scalar2=1.0 / WS, op0=ALU.mult, op1=ALU.mult)

        # ================= stage 2: experts =================
        with contextlib.ExitStack() as s2:
            gtp2 = s2.enter_context(tc.tile_pool(name="s2_gt", bufs=2))
            hidp = s2.enter_context(tc.tile_pool(name="hid", bufs=1))
            fcp = s2.enter_context(
                tc.tile_pool(name="fc_psum", bufs=3, space="PSUM"))
            prp = s2.enter_context(
                tc.tile_pool(name="pr_psum", bufs=3, space="PSUM"))

            for e in range(E):
                wfA = wfp.tile([P, CI, DFF // 2], BF16, tag="wf")
                nc.sync.dma_start(
                    out=wfA[:],
                    in_=fcw[e, :, 0:DFF // 2].rearrange("(o p) f -> p o f", p=P))
                wfB = wfp.tile([P, CI, DFF // 2], BF16, tag="wf")
                nc.sync.dma_start(
                    out=wfB[:],
                    in_=fcw[e, :, DFF // 2:DFF].rearrange("(o p) f -> p o f", p=P))
                wp8 = wpp.tile([P, KO2, 2, C], FP8, tag="wp")
                nc.sync.dma_start(
                    out=wp8[:],
                    in_=prw[e].rearrange("(a b p) n -> p a b n", b=2, p=P))
                fb = fcbp.tile([P, KO], F32, tag="fcb")
                nc.sync.dma_start(
                    out=fb[:], in_=fcb[e].rearrange("(o p) -> p o", p=P))
                hid8 = hidp.tile([P, KO2, 2, TH], FP8, tag="hid")

                # fc in bf16, th-major so pr can start after the first half
                for th in range(2):
                    for ko in range(KO):
                        wfh = wfA if ko < KO2 else wfB
                        kk = ko % KO2
                        fp = fcp.tile([P, 512], F32, tag="fp")
                        for ci in range(CI):
                            nc.tensor.matmul(
                                fp[:], wfh[:, ci, kk * P:(kk + 1) * P],
                                h2T[:, ci, th * 512:(th + 1) * 512],
                                start=(ci == 0), stop=(ci == CI - 1))
                        nc.scalar.activation(
                            out=hid8[:, ko // 2, ko % 2,
                                     th * 512:(th + 1) * 512],
                            in_=fp[:], func=ACTF.Gelu, bias=fb[:, ko:ko + 1])

                # pr in fp8 DoubleRow: contract 256 dff per pass
                for ti in range(NTT2):
                    ep0 = prp.tile([P, 512], F32, tag="ep")
                    ep1 = prp.tile([P, 512], F32, tag="ep")
                    eps2 = [ep0, ep1]
                    for ko2 in range(KO2):
                        for nh in range(2):
                            nc.tensor.matmul(
                                eps2[nh][:, :384],
                                hid8[:, ko2, :, ti * P:(ti + 1) * P],
                                wp8[:, ko2, :, nh * 384:(nh + 1) * 384],
                                start=(ko2 == 0), stop=(ko2 == KO2 - 1),
                                perf_mode=DR)
                    gt = gtp2.tile([P, C], F32, tag="gt2")
                    for nh in range(2):
                        nc.vector.tensor_scalar_mul(
                            gt[:, nh * 384:(nh + 1) * 384], eps2[nh][:, :384],
                            g_sb[:, ti, e:e + 1])
                    aw = nc.gpsimd.dma_start(
                        out=moe_dram[ti * P:(ti + 1) * P, :], in_=gt[:],
                        accum_op=ALU.add)
                    add_dep_helper(aw.ins, accum_tail[ti].ins, sync=True,
                                   reason="serialize accum")
                    accum_tail[ti] = aw

            od = nc.sync.dma_start(out=out_half[:], in_=moe_dram[:])
            for ti in range(NTT2):
                add_dep_helper(od.ins, accum_tail[ti].ins, sync=True,
                               reason="out after accum")

    _split_multi_waits(nc)
    return nc


def _get_nc():
    global _CACHED_NC
    if _CACHED_NC is None:
        _CACHED_NC = _build_nc()
    return _CACHED_NC


def _row_blocks(g):
    """Global row blocks (within a batch) owned by pair-rank g, in the local
    order produced by the chunked ReduceScatter."""
    return [slice(k * 512 + g * 256, k * 512 + (g + 1) * 256)
            for k in range(NCHUNK)]


def kernel(**inputs):
    np_in = {k: np.asarray(v) for k, v in inputs.items()}
    bf = lambda a: np.ascontiguousarray(a).astype(ml_dtypes.bfloat16)
    f32 = lambda a: np.ascontiguousarray(a, dtype=np.float32)

    x = f32(np_in["x"])
    attn_w = np_in["attn_w"]
    fcw = bf(np_in["fc_w"])
    prw8 = (f32(np_in["pr_w"]) * WS).astype(ml_dtypes.float8_e4m3)
    fcb = f32(np_in["fc_b"])
    gw = bf(np_in["gate_w"])
    ln1w, ln1b = f32(np_in["ln1_w"]), f32(np_in["ln1_b"])
    ln2w, ln2b = f32(np_in["ln2_w"]), f32(np_in["ln2_b"])
    proj_w = np_in["proj_w"]

    in_maps = []
    for core in range(NCORES):
        b, g = core // 2, core % 2
        cols = slice(g * HPC * HD, (g + 1) * HPC * HD)
        in_maps.append({
            "xb": x[b],
            "x_half": np.concatenate([x[b, blk] for blk in _row_blocks(g)]),
            "wq": bf(attn_w[:, cols]),
            "wk": bf(attn_w[:, C:2 * C][:, cols]),
            "wv": bf(attn_w[:, 2 * C:3 * C][:, cols]),
            "pw": bf(proj_w[cols, :]),
            "ln1w": ln1w, "ln1b": ln1b, "ln2w": ln2w, "ln2b": ln2b,
            "gw": gw, "fcw": fcw, "fcb": fcb, "prw": prw8,
        })

    nc = _get_nc()
    res = run_bass_kernel_spmd(nc, in_maps, core_ids=list(range(NCORES)))
    LAST_RESULT[0] = res

    out = np.empty((B, T, C), dtype=np.float32)
    for core in range(NCORES):
        b, g = core // 2, core % 2
        oh = res.results[core]["out_half"]
        for k, blk in enumerate(_row_blocks(g)):
            out[b, blk] = oh[k * 256:(k + 1) * 256]
    return out
